# revision 1
# baseline (speedup 1.0000x reference)
"""Trainium2 Bass kernel for nn_ContextualPositionEmbedding (B,H,S,D,NPOS = 2,16,2048,64,64).

out[b,h,i,j] = logits + interp(logits_int, pos) where
  gates = sigmoid(attn_logits + log(mask));  pos = clip(reverse-cumsum_j(gates), max 63)
  logits_int = query @ pos_emb;  interp = linear interpolation of logits_int at pos.

Sharding: batch*heads (32 pairs) split 4-per-core across 8 NeuronCores; each core
processes 64 independent [128 x 2048] row-tiles. pos_emb replicated. No collectives.

Algorithm per tile (exact where the validity flags pass; a host-side numpy fallback
covers anything else — never triggered for the target workload):
  - columns [0, JCUT): pos >= NPOS-1 there (checked by flag), so out = logits + f[r,63]
    (one scalar-engine pass with per-row bias).
  - strip [JCUT, 2048): pos is monotone-decreasing, dropping < 1 per column, so
    floor(pos) is a staircase crossing each level 63..1 exactly once. The per-element
    table gather is reconstructed by scattering the 63 table deltas (per-row-scaled
    int16) to the crossing columns (gpsimd local_scatter, per-partition indices) and
    prefix-scanning (tensor_tensor_scan, fp32 state). out = logits + g1 + w*g2.
"""

import numpy as np
from contextlib import ExitStack

import concourse.bass as bass
import concourse.tile as tile
from concourse import bacc, mybir
from concourse.bass_utils import run_bass_kernel_spmd

F32 = mybir.dt.float32
F16 = mybir.dt.float16
I32 = mybir.dt.int32
I16 = mybir.dt.int16
AF = mybir.ActivationFunctionType
OP = mybir.AluOpType

B, H, S, D, NPOS = 2, 16, 2048, 64, 64
N_CORES = 8
JCUT = 1888
WS = S - JCUT            # 160-wide exact strip
BH = B * H               # 32
BH_PER_CORE = BH // N_CORES   # 4
RB = S // 128            # 16 row-blocks per (b,h)
NT = BH_PER_CORE * RB    # 64 tiles per core
QSCALE = 30000.0         # per-row delta quantization target


def build_program(ntiles=NT, dbg=False):
    nc = bacc.Bacc("TRN2", target_bir_lowering=False, debug=False)
    attn = nc.dram_tensor("attn", [ntiles, 128, S], F32, kind="ExternalInput")
    qT = nc.dram_tensor("qT", [ntiles, 64, 128], F32, kind="ExternalInput")
    pe = nc.dram_tensor("pe", [D, NPOS], F32, kind="ExternalInput")
    iota = nc.dram_tensor("iota", [128, WS], I16, kind="ExternalInput")
    out = nc.dram_tensor("out", [ntiles, 128, S], F32, kind="ExternalOutput")
    flags = nc.dram_tensor("flags", [128, ntiles], F32, kind="ExternalOutput")
    if dbg:
        dbg_t = {
            name: nc.dram_tensor(f"dbg_{name}", [ntiles, 128, width], dt,
                                 kind="ExternalOutput")
            for name, width, dt in [
                ("u", WS, F32), ("w", WS, F32), ("drop", WS, F32),
                ("idx1", WS, I16), ("dst1", 64, I16), ("dbuf", 2 * WS, I16),
                ("S1", WS + 1, F32), ("S2", WS + 1, F32), ("g2", WS, F32),
                ("f", NPOS, F32), ("E", WS, F32), ("data2", 128, I16),
                ("scale", 1, F32),
            ]
        }

    with tile.TileContext(nc) as tc, ExitStack() as ctx:
        const_pool = ctx.enter_context(tc.tile_pool(name="const", bufs=1))
        big_pool = ctx.enter_context(tc.tile_pool(name="big", bufs=4))
        q_pool = ctx.enter_context(tc.tile_pool(name="q", bufs=3))
        psum_pool = ctx.enter_context(tc.tile_pool(name="ps", bufs=2, space="PSUM"))
        tb_pool = ctx.enter_context(tc.tile_pool(name="tb", bufs=3))
        s_pool = ctx.enter_context(tc.tile_pool(name="s", bufs=3))

        pe_sb = const_pool.tile([64, NPOS], F32)
        nc.sync.dma_start(pe_sb[:], pe.ap())
        iota_sb = const_pool.tile([128, WS], I16)
        nc.sync.dma_start(iota_sb[:], iota.ap())
        zi16 = const_pool.tile([128, WS], I16)
        nc.vector.memset(zi16[:], 0)
        flags_sb = const_pool.tile([128, ntiles], F32)

        for t in range(ntiles):
            # ---- load
            lg = big_pool.tile([128, S], F32, tag="lg")
            nc.sync.dma_start(lg[:], attn.ap()[t])
            qt = q_pool.tile([64, 128], F32, tag="qt")
            nc.sync.dma_start(qt[:], qT.ap()[t])

            # ---- table f = q @ pos_emb  [128, 64]
            fps = psum_pool.tile([128, NPOS], F32, tag="fps")
            nc.tensor.matmul(fps[:], lhsT=qt[:], rhs=pe_sb[:], start=True, stop=True)
            f = tb_pool.tile([128, NPOS], F32, tag="f")
            nc.scalar.activation(f[:], fps[:], AF.Copy)

            # ---- negated deltas + per-row int16 quantization
            nd = tb_pool.tile([128, NPOS], F32, tag="nd")
            nc.vector.memset(nd[:, 63:64], 0.0)
            nc.vector.tensor_sub(nd[:, 0:63], f[:, 0:63], f[:, 1:64])
            ndmax = tb_pool.tile([128, 1], F32, tag="ndmax")
            nc.vector.tensor_reduce(ndmax[:], nd[:], mybir.AxisListType.X, OP.max,
                                    apply_absolute_value=True)
            ndmax_s = tb_pool.tile([128, 1], F32, tag="ndmax_s")
            nc.vector.tensor_scalar(ndmax_s[:], ndmax[:], 1e-6, None, OP.max)
            recip = tb_pool.tile([128, 1], F32, tag="recip")
            nc.vector.reciprocal(recip[:], ndmax_s[:])
            scale = tb_pool.tile([128, 1], F32, tag="scale")
            nc.vector.tensor_scalar(scale[:], recip[:], QSCALE, None, OP.mult)
            inv = tb_pool.tile([128, 1], F32, tag="inv")
            nc.vector.tensor_scalar(inv[:], ndmax_s[:], 1.0 / QSCALE, None, OP.mult)
            sneg = tb_pool.tile([128, 1], F32, tag="sneg")
            nc.vector.tensor_scalar(sneg[:], scale[:], -1.0, None, OP.mult)
            data2 = tb_pool.tile([128, 128], I16, tag="data2")
            nc.vector.tensor_scalar(data2[:, 0:64], nd[:], scale[:], None, OP.mult)
            # ndd gets its own per-row scale (|ndd| can reach 2*ndmax)
            ndd = tb_pool.tile([128, 63], F32, tag="ndd")
            nc.vector.tensor_sub(ndd[:], nd[:, 1:64], nd[:, 0:63])
            nddmax = tb_pool.tile([128, 1], F32, tag="nddmax")
            nc.vector.tensor_reduce(nddmax[:], ndd[:], mybir.AxisListType.X, OP.max,
                                    apply_absolute_value=True)
            nddmax_s = tb_pool.tile([128, 1], F32, tag="nddmax_s")
            nc.vector.tensor_scalar(nddmax_s[:], nddmax[:], 1e-6, None, OP.max)
            recip2 = tb_pool.tile([128, 1], F32, tag="recip2")
            nc.vector.reciprocal(recip2[:], nddmax_s[:])
            scale2 = tb_pool.tile([128, 1], F32, tag="scale2")
            nc.vector.tensor_scalar(scale2[:], recip2[:], QSCALE, None, OP.mult)
            inv2 = tb_pool.tile([128, 1], F32, tag="inv2")
            nc.vector.tensor_scalar(inv2[:], nddmax_s[:], 1.0 / QSCALE, None, OP.mult)
            nc.vector.tensor_scalar(data2[:, 64:127], ndd[:], scale2[:], None, OP.mult)
            nc.vector.memset(data2[:, 127:128], 0)

            # ---- strip: gates (shifted by one col), exclusive prefix scan, pos
            gp = s_pool.tile([128, WS + 1], F32, tag="gp")
            nc.vector.memset(gp[:, 0:1], 0.0)
            nc.scalar.activation(gp[:, 1:WS + 1], lg[:, JCUT:S], AF.Sigmoid)
            E = s_pool.tile([128, WS], F32, tag="E")
            nc.vector.tensor_tensor_scan(E[:], gp[:, 0:WS], gp[:, 0:WS],
                                         0.0, OP.add, OP.max)
            T = s_pool.tile([128, 1], F32, tag="T")
            nc.vector.tensor_add(T[:], E[:, WS - 1:WS], gp[:, WS:WS + 1])
            u = s_pool.tile([128, WS], F32, tag="u")
            nc.vector.tensor_scalar(u[:], E[:], T[:], -63.0, OP.subtract, OP.max)

            # flf_neg = ceil(u) = -floor(pos), robust to HW convert rounding:
            # conv = int(u) under trunc/RNE/floor; ceil(u) = conv + (conv < u)
            ifl = s_pool.tile([128, WS], I32, tag="ifl")
            nc.vector.tensor_copy(ifl[:], u[:])
            convf = s_pool.tile([128, WS], F32, tag="convf")
            nc.vector.tensor_copy(convf[:], ifl[:])
            corr = s_pool.tile([128, WS], F32, tag="corr")
            nc.vector.tensor_tensor(corr[:], convf[:], u[:], OP.is_lt)
            flfp = s_pool.tile([128, WS + 1], F32, tag="flfp")
            nc.vector.memset(flfp[:, WS:WS + 1], 0.0)
            nc.vector.tensor_add(flfp[:, 0:WS], convf[:], corr[:])
            w = s_pool.tile([128, WS], F32, tag="w")
            nc.vector.tensor_sub(w[:], flfp[:, 0:WS], u[:])        # pos - floor(pos)
            drop = s_pool.tile([128, WS], F32, tag="drop")
            nc.vector.tensor_sub(drop[:], flfp[:, 1:WS + 1], flfp[:, 0:WS])
            tdf = s_pool.tile([128, WS], F32, tag="tdf")
            nc.vector.scalar_tensor_tensor(tdf[:], drop[:], -1.0, flfp[:, 0:WS],
                                           OP.mult, OP.mult)       # drop*fl
            idx1 = s_pool.tile([128, WS], I16, tag="idx1")
            nc.vector.tensor_scalar(idx1[:], tdf[:], -1.0, 63.0, OP.add, OP.min)

            # ---- scatter 1: drop-column (iota value = col+1) per level slot
            dst1 = s_pool.tile([128, 64], I16, tag="dst1")
            nc.gpsimd.local_scatter(dst1[:], iota_sb[:], idx1[:],
                                    channels=128, num_elems=64, num_idxs=WS)
            # ---- scatter 2: quantized deltas to drop columns
            idx2 = s_pool.tile([128, 128], I16, tag="idx2")
            nc.vector.tensor_scalar(idx2[:, 0:64], dst1[:], -1.0, None, OP.add)
            nc.vector.tensor_scalar(idx2[:, 64:127], dst1[:, 0:63], float(WS - 1),
                                    None, OP.add)
            nc.vector.memset(idx2[:, 127:128], -1)
            dbuf = s_pool.tile([128, 2 * WS], I16, tag="dbuf")
            nc.gpsimd.local_scatter(dbuf[:], data2[:], idx2[:],
                                    channels=128, num_elems=2 * WS, num_idxs=128)

            # ---- delta prefix scans (fp32 state over int16)
            S1 = s_pool.tile([128, WS + 1], F32, tag="S1")
            nc.vector.memset(S1[:, 0:1], 0.0)
            nc.vector.tensor_tensor_scan(S1[:, 1:WS + 1], dbuf[:, 0:WS], zi16[:],
                                         0.0, OP.add, OP.add)
            S2 = s_pool.tile([128, WS + 1], F32, tag="S2")
            nc.vector.memset(S2[:, 0:1], 0.0)
            nc.vector.tensor_tensor_scan(S2[:, 1:WS + 1], dbuf[:, WS:2 * WS], zi16[:],
                                         0.0, OP.add, OP.add)
            C1 = s_pool.tile([128, 1], F32, tag="C1")
            nc.vector.tensor_scalar(C1[:], f[:, 0:1], sneg[:], S1[:, WS:WS + 1],
                                    OP.mult, OP.add)
            C2 = s_pool.tile([128, 1], F32, tag="C2")
            nc.vector.tensor_scalar(C2[:], nd[:, 0:1], scale2[:], S2[:, WS:WS + 1],
                                    OP.mult, OP.add)

            # ---- fast path: out[:, :JCUT] = logits + f63   (in-place on lg)
            nc.scalar.activation(lg[:, 0:JCUT], lg[:, 0:JCUT], AF.Identity,
                                 bias=f[:, 63:64], scale=1.0)

            # ---- strip combine: out = logits + g1 + w*g2
            g1 = s_pool.tile([128, WS], F32, tag="g1")
            nc.vector.tensor_scalar(g1[:], S1[:, 0:WS], C1[:], inv[:],
                                    OP.subtract, OP.mult)
            g2 = s_pool.tile([128, WS], F32, tag="g2")
            nc.vector.tensor_scalar(g2[:], S2[:, 0:WS], C2[:], inv2[:],
                                    OP.subtract, OP.mult)
            nc.vector.tensor_add(lg[:, JCUT:S], lg[:, JCUT:S], g1[:])
            wg2 = s_pool.tile([128, WS], F32, tag="wg2")
            nc.vector.tensor_mul(wg2[:], w[:], g2[:])
            nc.vector.tensor_add(lg[:, JCUT:S], lg[:, JCUT:S], wg2[:])

            # ---- flags: pos[JCUT] >= 63 AND all 63 levels deposited
            posok = s_pool.tile([128, 1], F32, tag="posok")
            nc.vector.tensor_scalar(posok[:], T[:], 63.0, None, OP.is_ge)
            rmin = s_pool.tile([128, 1], I16, tag="rmin")
            nc.vector.tensor_reduce(rmin[:], dst1[:, 0:63], mybir.AxisListType.X, OP.min)
            levok = s_pool.tile([128, 1], F32, tag="levok")
            nc.vector.tensor_scalar(levok[:], rmin[:], 0.5, None, OP.is_ge)
            nc.vector.tensor_mul(flags_sb[:, t:t + 1], posok[:], levok[:])

            if dbg:
                for name, ap_ in [("u", u[:]), ("w", w[:]), ("drop", drop[:]),
                                  ("idx1", idx1[:]), ("dst1", dst1[:]),
                                  ("dbuf", dbuf[:]), ("S1", S1[:]), ("S2", S2[:]),
                                  ("g2", g2[:]), ("f", f[:]), ("E", E[:]),
                                  ("data2", data2[:]), ("scale", scale[:])]:
                    nc.sync.dma_start(dbg_t[name].ap()[t], ap_)

            # ---- store
            nc.sync.dma_start(out.ap()[t], lg[:])

        nc.sync.dma_start(flags.ap(), flags_sb[:])

    nc.compile()
    return nc


_PROG_CACHE = {}


def _get_program(ntiles=NT):
    if ntiles not in _PROG_CACHE:
        _PROG_CACHE[ntiles] = build_program(ntiles)
    return _PROG_CACHE[ntiles]


def _prep_core_inputs(attn_f32, qT_all, pe2d, iota_np):
    """attn_f32: [BH, S, S]; qT_all: [BH, D, S]. Returns list of 8 in_maps."""
    in_maps = []
    for c in range(N_CORES):
        sl = slice(c * BH_PER_CORE, (c + 1) * BH_PER_CORE)
        a = attn_f32[sl].reshape(NT, 128, S)
        q = np.ascontiguousarray(
            qT_all[sl].reshape(BH_PER_CORE, D, RB, 128).transpose(0, 2, 1, 3)
        ).reshape(NT, D, 128)
        in_maps.append({"attn": np.ascontiguousarray(a), "qT": q,
                        "pe": pe2d, "iota": iota_np})
    return in_maps


def _reference_fallback(query, attn_logits, mask, pos_emb):
    logits = attn_logits + np.log(mask)
    gates = 1.0 / (1.0 + np.exp(-logits))
    pos = np.cumsum(gates[..., ::-1], axis=-1)[..., ::-1]
    pos = np.minimum(pos, np.float32(NPOS - 1))
    pos_ceil = np.ceil(pos).astype(np.int32)
    pos_floor = np.floor(pos).astype(np.int32)
    logits_int = np.einsum('bhsd,dn->bhsn', query, pos_emb[0, 0])
    lc = np.take_along_axis(logits_int, pos_ceil, axis=-1)
    lf = np.take_along_axis(logits_int, pos_floor, axis=-1)
    w = pos - pos_floor.astype(pos.dtype)
    return (logits + lc * w + lf * (1.0 - w)).astype(np.float32)


def run_on_device(inputs, trace=False):
    """Returns (out [B,H,S,S] f32, flags_ok bool, BassKernelResults)."""
    query = np.asarray(inputs["query"], np.float32)
    attn_logits = np.asarray(inputs["attn_logits"], np.float32)
    pos_emb = np.asarray(inputs["pos_emb"], np.float32)

    attn_f32 = attn_logits.reshape(BH, S, S)
    qT_all = np.ascontiguousarray(query.reshape(BH, S, D).transpose(0, 2, 1))
    pe2d = np.ascontiguousarray(pos_emb.reshape(D, NPOS))
    iota_np = np.broadcast_to(
        np.arange(1, WS + 1, dtype=np.int16), (128, WS)).copy()

    nc = _get_program(NT)
    in_maps = _prep_core_inputs(attn_f32, qT_all, pe2d, iota_np)
    res = run_bass_kernel_spmd(nc, in_maps, core_ids=list(range(N_CORES)),
                               trace=trace)
    outs = [res.results[c]["out"] for c in range(N_CORES)]
    fl = [res.results[c]["flags"] for c in range(N_CORES)]
    out = np.concatenate(outs, axis=0).reshape(B, H, S, S)
    flags_ok = all(np.all(f >= 0.5) for f in fl)
    return out, flags_ok, res


def kernel(query, attn_logits, mask, pos_emb):
    query = np.asarray(query)
    attn_logits = np.asarray(attn_logits)
    mask = np.asarray(mask)
    pos_emb = np.asarray(pos_emb)
    if not np.all(mask == 1.0):
        return _reference_fallback(
            query.astype(np.float32), attn_logits.astype(np.float32),
            mask.astype(np.float32), pos_emb.astype(np.float32))
    out, flags_ok, _ = run_on_device(
        {"query": query, "attn_logits": attn_logits, "pos_emb": pos_emb})
    if not flags_ok or not np.isfinite(out).all():
        return _reference_fallback(
            query.astype(np.float32), attn_logits.astype(np.float32),
            mask.astype(np.float32), pos_emb.astype(np.float32))
    return out

